# revision 14
# baseline (speedup 1.0000x reference)
"""Graphormer3D encoder layer on 8 Trainium2 NeuronCores.

Sharding: data-parallel over the 16 graphs (2 per core); params replicated.
On-chip layout is feature-major (x^T: [feature, token]) in fp16 with fp32 PSUM
accumulation:
  - LayerNorm affine (g,b) folded into the following weight matrices on host;
    mean/var via ones-matmul partition reductions on TensorE, rsqrt as
    exp(-0.5*ln(var)).
  - QKV emitted per-head and interleaved with that head's attention so the
    tensor queue never drains at the phase boundary.
  - Attention computed transposed: scores^T = (k-slice as stationary) @ q, so
    probs come out in [k_tok, q] layout and need no transposes.
    exp(score+bias) = exp(score)*exp(bias): exp(bias)^T precomputed on host,
    applied with one DVE multiply; exp(score) evacuates the PSUM on ScalarE.
  - v^T gets a ones column appended (K=1 matmuls into the transpose PSUM) so
    the attn^T matmul also produces the softmax denominators; normalization
    happens at PSUM evacuation via a K=1 broadcast matmul of 1/sums.
  - out-proj / LN2 / FFN are split per graph (the two token halves) so graph
    0's FFN overlaps graph 1's attention drain.
All matmuls use N<=512 so each accumulation group stays in one PSUM bank.
"""
import numpy as np

N_NODE, N_GRAPH, D = 512, 16, 768
H, HD, FFN = 8, 96, 3072
EPS = 1e-5
NC = 8            # cores
G = 2             # graphs per core
T = G * N_NODE    # tokens per core (1024)
KC = D // 128     # 6 feature chunks
FC = FFN // 128   # 24 ffn chunks
NQT = N_NODE // 128  # 4 q/k tiles per graph
HLF = (slice(0, 512), slice(512, 1024))
VW = HD + 1       # v^T columns incl. the ones column (97)
VS = HD + 2       # v^T tile stride, kept even for PSUM 4-byte alignment

_cached = {}


def _build():
    import concourse.bass as bass
    import concourse.mybir as mybir
    import concourse.tile as tile
    import concourse.bacc as bacc
    from contextlib import ExitStack

    F16 = mybir.dt.float16
    F32 = mybir.dt.float32
    AF = mybir.ActivationFunctionType
    OP = mybir.AluOpType

    nc = bacc.Bacc("TRN2", target_bir_lowering=False, debug=False, num_devices=NC)

    di = lambda name, shape, dt: nc.declare_dram_parameter(name, shape, dt, isOutput=False)
    xt_d = di("xt", [KC, 128, T], F16)
    bias_d = di("biasb", [G * H, N_NODE, N_NODE], F16)   # exp(bias)^T per gh
    mask_d = di("maskrow", [1, T], F16)
    wqkv_d = di("wqkv", [KC, 128, 3 * D], F16)
    bqkv_d = di("bqkv", [HD, 3 * H], F32)
    wout_d = di("wout", [H, HD, D], F16)
    bout_d = di("bout", [128, KC], F32)
    wfc1_d = di("wfc1", [KC, 128, FFN], F16)
    bfc1_d = di("bfc1", [128, FC], F32)
    wfc2_d = di("wfc2", [FC, 128, D], F16)
    bfc2_d = di("bfc2", [128, KC], F32)
    ident_d = di("ident", [128, 128], F16)
    ones_d = di("ones", [128, 128], F16)
    yt_d = nc.declare_dram_parameter("yt", [KC, 128, T], F16, isOutput=True)

    with tile.TileContext(nc) as tc, ExitStack() as top:
        const = top.enter_context(tc.tile_pool(name="const", bufs=1))
        # persistent activation pools (LIFO pool stack: opens/closes nest)
        h_pool = top.enter_context(tc.tile_pool(name="h", bufs=KC))
        y1_pool = top.enter_context(tc.tile_pool(name="y1", bufs=KC))
        stat_pool = top.enter_context(tc.tile_pool(name="stat", bufs=1))
        sq_pool = top.enter_context(tc.tile_pool(name="sq", bufs=2))
        tmp_pool = top.enter_context(tc.tile_pool(name="tmp", bufs=2))
        yo_pool = top.enter_context(tc.tile_pool(name="yo", bufs=2))
        s_x = ExitStack()
        x_pool = s_x.enter_context(tc.tile_pool(name="x", bufs=KC))
        # x DMAs issued first so LN1 stats can start ASAP
        x_tiles = []
        for k in range(KC):
            xt = x_pool.tile([128, T], F16, tag="x")
            nc.sync.dma_start(xt[:], xt_d[k])
            x_tiles.append(xt)

        def load_const(name, dram, shape, dt):
            t = const.tile(shape, dt, tag=name)
            nc.sync.dma_start(t[:], dram[:])
            return t

        ident = load_const("ident", ident_d, [128, 128], F16)
        ones = load_const("ones", ones_d, [128, 128], F16)
        mask_sb = load_const("mask", mask_d, [1, T], F16)
        bqkv = load_const("bqkv", bqkv_d, [HD, 3 * H], F32)
        bout = load_const("bout", bout_d, [128, KC], F32)
        bfc1 = load_const("bfc1", bfc1_d, [128, FC], F32)
        bfc2 = load_const("bfc2", bfc2_d, [128, KC], F32)
        eps_sb = const.tile([128, 1], F32, tag="eps")
        nc.vector.memset(eps_sb[:], EPS)
        warm = const.tile([1, 1], F32, tag="warm")
        for fn in (AF.Identity, AF.Ln, AF.Exp, AF.Copy, AF.Gelu):
            nc.scalar.activation(warm[:], eps_sb[0:1, :], fn)

        s_attn = ExitStack()
        attn_pool = s_attn.enter_context(tc.tile_pool(name="attn", bufs=1))
        attn_sb = attn_pool.tile([HD, H * T], F16, tag="attn")
        s_wout = ExitStack()
        wout_pool = s_wout.enter_context(tc.tile_pool(name="wout", bufs=H))
        s_qkv = ExitStack()
        qkv_pool = s_qkv.enter_context(tc.tile_pool(name="qkv", bufs=1))
        q_sb = qkv_pool.tile([HD, H * T], F16, tag="q")
        k_sb = qkv_pool.tile([HD, H * T], F16, tag="k")
        v_sb = qkv_pool.tile([VS, H * T], F16, tag="v")  # row HD is all-ones
        nc.vector.memset(v_sb[HD:VS, :], 1.0)
        qkv_sbs = [q_sb, k_sb, v_sb]
        wqkv_pool = s_qkv.enter_context(tc.tile_pool(name="wqkv", bufs=KC))
        wq_tiles = []
        for k in range(KC):
            wt = wqkv_pool.tile([128, 3 * D], F16, tag="wqkv")
            nc.sync.dma_start(wt[:], wqkv_d[k])
            wq_tiles.append(wt)
        wo_tiles = [wout_pool.tile([HD, D], F16, tag="wout", name=f"wo_{i}")
                    for i in range(H)]

        def layer_norm_stats(x_sl, sq_sl, psum_pool, sl, n_sl):
            """feature-major LN stats over the partition axis for token slice
            sl; returns (mu16, rs) broadcast tiles of width n_sl."""
            nh = (n_sl + 511) // 512
            ps_s = psum_pool.tile([128, n_sl], F32, tag="st")
            for k in range(KC):
                for c in range(nh):
                    cs = slice(c * 512, min((c + 1) * 512, n_sl))
                    nc.tensor.matmul(ps_s[:, cs], ones[:], x_sl[k][:, cs],
                                     start=(k == 0), stop=(k == KC - 1))
            ps_q = psum_pool.tile([128, n_sl], F32, tag="st")
            for k in range(KC):
                for c in range(nh):
                    cs = slice(c * 512, min((c + 1) * 512, n_sl))
                    nc.tensor.matmul(ps_q[:, cs], ones[:], sq_sl[k][:, cs],
                                     start=(k == 0), stop=(k == KC - 1))
            mu = stat_pool.tile([128, n_sl], F32, tag="mu")
            nc.vector.tensor_scalar_mul(mu[:], ps_s[:], 1.0 / D)
            ms = stat_pool.tile([128, n_sl], F32, tag="ms")
            nc.vector.tensor_scalar_mul(ms[:], ps_q[:], 1.0 / D)
            var = stat_pool.tile([128, n_sl], F32, tag="var")
            nc.vector.tensor_tensor(var[:], mu[:], mu[:], op=OP.mult)
            nc.vector.tensor_tensor(var[:], ms[:], var[:], op=OP.subtract)
            nc.scalar.activation(ms[:], var[:], AF.Ln, bias=eps_sb[:])
            rs = stat_pool.tile([128, n_sl], F16, tag="rs")
            nc.scalar.activation(rs[:], ms[:], AF.Exp, scale=-0.5)
            mu16 = stat_pool.tile([128, n_sl], F16, tag="mu16")
            nc.vector.tensor_copy(mu16[:], mu[:])
            return mu16, rs

        # ---------------- phase 1: LN1 stats + h1 + mask bcast ----------------
        with tc.tile_pool(name="ps_st", bufs=3, space="PSUM") as ps_st:
            sq_tiles = []
            for k in range(KC):
                sq = sq_pool.tile([128, T], F16, tag="sq")
                nc.vector.tensor_tensor(sq[:], x_tiles[k][:], x_tiles[k][:], op=OP.mult)
                sq_tiles.append(sq)
            mu16, rs = layer_norm_stats([x[:] for x in x_tiles],
                                        [s[:] for s in sq_tiles], ps_st,
                                        slice(0, T), T)
            h1 = []
            for k in range(KC):
                ht = h_pool.tile([128, T], F16, tag="h")
                nc.vector.tensor_tensor(ht[:], x_tiles[k][:], mu16[:], op=OP.subtract)
                nc.vector.tensor_tensor(ht[:], ht[:], rs[:], op=OP.mult)
                h1.append(ht)
            # mask broadcast [128, T]
            ps_m = ps_st.tile([128, T], F32, tag="st")
            for hf in range(2):
                nc.tensor.matmul(ps_m[:, HLF[hf]], ones[0:1, :], mask_sb[:, HLF[hf]],
                                 start=True, stop=True)
            mask_b = stat_pool.tile([128, T], F16, tag="maskb")
            nc.vector.tensor_copy(mask_b[:], ps_m[:])

        # ---------------- phase 2: per-head QKV + attention ----------------
        with tc.tile_pool(name="biasbuf", bufs=10) as bias_pool, \
             tc.tile_pool(name="probs", bufs=8) as prob_pool, \
             tc.tile_pool(name="vt", bufs=2) as vt_pool, \
             tc.tile_pool(name="small", bufs=2) as small_pool, \
             tc.tile_pool(name="ps_qkv", bufs=2, space="PSUM") as ps_qkv, \
             tc.tile_pool(name="ps_sc", bufs=2, space="PSUM") as ps_sc, \
             tc.tile_pool(name="ps_vt", bufs=1, space="PSUM") as ps_vt, \
             tc.tile_pool(name="ps_at", bufs=2, space="PSUM") as ps_at, \
             tc.tile_pool(name="ps_bc", bufs=1, space="PSUM") as ps_bc:
            for hh in range(H):
                # QKV for this head: q, k, v chunks [HD, T]
                for tau in range(3):
                    th = tau * H + hh
                    for hf in range(2):
                        ps = ps_qkv.tile([HD, 512], F32, tag="qkv")
                        for k in range(KC):
                            nc.tensor.matmul(
                                ps[:], wq_tiles[k][:, th * HD:(th + 1) * HD],
                                h1[k][:, HLF[hf]], start=(k == 0), stop=(k == KC - 1))
                        dst = qkv_sbs[tau][0:HD, hh * T + hf * 512: hh * T + hf * 512 + 512]
                        nc.scalar.activation(dst, ps[:], AF.Identity, bias=bqkv[:, th:th + 1])
                for g in range(G):
                    gh = g * H + hh
                    base = hh * T + g * N_NODE
                    # scores^T per k-tile; probs = exp(sc)*exp(bias)
                    p_tiles = []
                    for kt in range(NQT):
                        bt = bias_pool.tile([128, N_NODE], F16, tag="bias")
                        nc.sync.dma_start(bt[:], bias_d[gh, kt * 128:(kt + 1) * 128, :])
                        sc = ps_sc.tile([128, N_NODE], F32, tag="sc")
                        nc.tensor.matmul(sc[:], k_sb[:, base + kt * 128: base + (kt + 1) * 128],
                                         q_sb[:, base: base + N_NODE], start=True, stop=True)
                        praw = prob_pool.tile([128, N_NODE], F16, tag="praw")
                        nc.scalar.activation(praw[:], sc[:], AF.Exp)
                        p = prob_pool.tile([128, N_NODE], F16, tag="p")
                        nc.vector.tensor_tensor(p[:], praw[:], bt[:], op=OP.mult)
                        p_tiles.append(p)
                    # v^T per k-tile (ones column comes from v_sb's ones row)
                    vtp = ps_vt.tile([128, NQT * VS], F16, tag="vt")
                    for kt in range(NQT):
                        nc.tensor.transpose(vtp[:, kt * VS: (kt + 1) * VS],
                                            v_sb[:, base + kt * 128: base + (kt + 1) * 128],
                                            ident[0:VS, 0:VS])
                    vt = vt_pool.tile([128, NQT * VS], F16, tag="vt")
                    nc.vector.tensor_copy(vt[:], vtp[:])
                    # attn^T (+ sums row) = [v^T | 1]^T @ probs^T
                    pa = ps_at.tile([VW, N_NODE], F32, tag="at")
                    for kt in range(NQT):
                        nc.tensor.matmul(pa[:], vt[:, kt * VS: kt * VS + VW], p_tiles[kt][:],
                                         start=(kt == 0), stop=(kt == NQT - 1))
                    # normalize: 1/sums broadcast via K=1 matmul
                    s32 = small_pool.tile([1, N_NODE], F32, tag="s32")
                    nc.scalar.activation(s32[:], pa[HD:VW, :], AF.Copy)
                    r32 = small_pool.tile([1, N_NODE], F32, tag="r32")
                    nc.vector.reciprocal_approx_fast(out=r32[:], in_=s32[:])
                    r16 = small_pool.tile([1, N_NODE], F16, tag="r16")
                    nc.scalar.activation(r16[:], r32[:], AF.Copy)
                    pb = ps_bc.tile([HD, N_NODE], F32, tag="bc")
                    nc.tensor.matmul(pb[:], ones[0:1, 0:HD], r16[:], start=True, stop=True)
                    dst = attn_sb[:, base: base + N_NODE]
                    nc.scalar.activation(dst, pa[0:HD, :], AF.Copy)
                    nc.vector.tensor_tensor(dst, pb[:], dst, op=OP.mult)
            for hh in range(H):
                nc.sync.dma_start(wo_tiles[hh][:], wout_d[hh])
        s_qkv.close()

        # ---------------- phase 3a: per-graph out-proj + residual ----------------
        y1_tiles = [y1_pool.tile([128, T], F16, tag="y1", name=f"y1_{i}") for i in range(KC)]
        with tc.tile_pool(name="ps_c", bufs=2, space="PSUM") as ps_c:
            for hf in range(2):
                for m in range(KC):
                    po = ps_c.tile([128, 512], F32, tag="mm")
                    for hh in range(H):
                        nc.tensor.matmul(po[:], wo_tiles[hh][:, m * 128:(m + 1) * 128],
                                         attn_sb[:, hh * T + hf * 512: hh * T + hf * 512 + 512],
                                         start=(hh == 0), stop=(hh == H - 1))
                    t = tmp_pool.tile([128, 512], F16, tag="tmp")
                    nc.vector.scalar_tensor_tensor(t[:], po[:], bout[:, m:m + 1],
                                                   mask_b[:, HLF[hf]],
                                                   op0=OP.add, op1=OP.mult)
                    nc.vector.tensor_tensor(y1_tiles[m][:, HLF[hf]], t[:],
                                            x_tiles[m][:, HLF[hf]], op=OP.add)
        s_wout.close()
        s_attn.close()
        s_x.close()

        # ---------------- phase 3b: LN2 + FFN ----------------
        with tc.tile_pool(name="ps_f", bufs=2, space="PSUM") as ps_f, \
             tc.tile_pool(name="ps_st2", bufs=2, space="PSUM") as ps_st2, \
             tc.tile_pool(name="wfc1", bufs=KC) as wfc1_pool, \
             tc.tile_pool(name="gelu", bufs=FC) as gelu_pool:
            wf1_tiles = []
            for k in range(KC):
                wt = wfc1_pool.tile([128, FFN], F16, tag="wfc1")
                nc.sync.dma_start(wt[:], wfc1_d[k])
                wf1_tiles.append(wt)

            # LN2 stats + h2, per half
            h2 = [h_pool.tile([128, T], F16, tag="h", name=f"h2_{i}") for i in range(KC)]
            for hf in range(2):
                sq2 = []
                for k in range(KC):
                    sq = sq_pool.tile([128, 512], F16, tag="sq")
                    nc.vector.tensor_tensor(sq[:], y1_tiles[k][:, HLF[hf]],
                                            y1_tiles[k][:, HLF[hf]], op=OP.mult)
                    sq2.append(sq)
                mu16, rs = layer_norm_stats(
                    [y[:, HLF[hf]] for y in y1_tiles], [s[:] for s in sq2],
                    ps_st2, HLF[hf], 512)
                for k in range(KC):
                    nc.vector.tensor_tensor(h2[k][:, HLF[hf]], y1_tiles[k][:, HLF[hf]],
                                            mu16[:], op=OP.subtract)
                    nc.vector.tensor_tensor(h2[k][:, HLF[hf]], h2[k][:, HLF[hf]],
                                            rs[:], op=OP.mult)

            gelu_tiles = [gelu_pool.tile([128, T], F16, tag="gelu", name=f"gelu_{i}") for i in range(FC)]
            for n in range(FC):
                pf = ps_f.tile([128, T], F32, tag="mm")
                for hf in range(2):
                    for k in range(KC):
                        nc.tensor.matmul(pf[:, HLF[hf]], wf1_tiles[k][:, n * 128:(n + 1) * 128],
                                         h2[k][:, HLF[hf]], start=(k == 0), stop=(k == KC - 1))
                nc.scalar.activation(gelu_tiles[n][:], pf[:], AF.Gelu,
                                     bias=bfc1[:, n:n + 1])

            with tc.tile_pool(name="wfc2", bufs=FC) as wfc2_pool:
                wf2_tiles = []
                for kk in range(FC):
                    wt = wfc2_pool.tile([128, D], F16, tag="wfc2")
                    nc.sync.dma_start(wt[:], wfc2_d[kk])
                    wf2_tiles.append(wt)
                for m in range(KC):
                    py = ps_f.tile([128, T], F32, tag="mm")
                    for hf in range(2):
                        for kk in range(FC):
                            nc.tensor.matmul(py[:, HLF[hf]], wf2_tiles[kk][:, m * 128:(m + 1) * 128],
                                             gelu_tiles[kk][:, HLF[hf]],
                                             start=(kk == 0), stop=(kk == FC - 1))
                    yo = yo_pool.tile([128, T], F16, tag="yo")
                    nc.vector.scalar_tensor_tensor(yo[:], py[:], bfc2[:, m:m + 1],
                                                   y1_tiles[m][:], op0=OP.add, op1=OP.add)
                    nc.sync.dma_start(yt_d[m], yo[:])

    nc.compile()
    return nc


def _get_runner():
    if "runner" in _cached:
        return _cached["runner"]
    import jax
    from jax.sharding import Mesh, PartitionSpec
    from jax.experimental.shard_map import shard_map
    import concourse.mybir as mybir
    from concourse.bass2jax import _bass_exec_p, install_neuronx_cc_hook, partition_id_tensor

    nc = _build()
    install_neuronx_cc_hook()
    partition_name = nc.partition_id_tensor.name if nc.partition_id_tensor else None
    in_names, out_names, out_avals, zero_outs = [], [], [], []
    for alloc in nc.m.functions[0].allocations:
        if not isinstance(alloc, mybir.MemoryLocationSet):
            continue
        name = alloc.memorylocations[0].name
        if alloc.kind == "ExternalInput":
            if name != partition_name:
                in_names.append(name)
        elif alloc.kind == "ExternalOutput":
            out_names.append(name)
            shape = tuple(alloc.tensor_shape)
            dtype = mybir.dt.np(alloc.dtype)
            out_avals.append(jax.core.ShapedArray(shape, dtype))
            zero_outs.append(np.zeros(shape, dtype))
    n_params = len(in_names)
    all_in_names = in_names + out_names + ([partition_name] if partition_name else [])

    def _body(*args):
        operands = list(args)
        if partition_name is not None:
            operands.append(partition_id_tensor())
        outs = _bass_exec_p.bind(
            *operands,
            out_avals=tuple(out_avals),
            in_names=tuple(all_in_names),
            out_names=tuple(out_names),
            lowering_input_output_aliases=(),
            sim_require_finite=False,
            sim_require_nnan=False,
            nc=nc,
        )
        return tuple(outs)

    donate = tuple(range(n_params, n_params + len(out_avals)))
    devices = jax.devices()[:NC]
    mesh = Mesh(np.asarray(devices), ("core",))
    in_specs = (PartitionSpec("core"),) * (n_params + len(out_avals))
    out_specs = (PartitionSpec("core"),) * len(out_names)
    sharded = jax.jit(
        shard_map(_body, mesh=mesh, in_specs=in_specs, out_specs=out_specs, check_rep=False),
        donate_argnums=donate, keep_unused=True,
    )

    runner = {
        "nc": nc, "sharded": sharded, "in_names": in_names,
        "out_names": out_names, "out_avals": out_avals, "zero_outs": zero_outs,
    }
    _cached["runner"] = runner
    return runner


def prep_inputs(x, attn_bias, node_non_padding_mask, in_w, in_b, out_w, out_b,
                ln1_g, ln1_b, fc1_w, fc1_b, fc2_w, fc2_b, ln2_g, ln2_b):
    """Host-side sharding/layout prep. Returns per-core dicts keyed by dram
    parameter name."""
    f16, f32 = np.float16, np.float32
    x = np.asarray(x, f32)
    xt = x.transpose(2, 1, 0).reshape(D, N_GRAPH * N_NODE).astype(f16)  # [768, 8192]
    xt_pc = [np.ascontiguousarray(xt[:, c * T:(c + 1) * T]).reshape(KC, 128, T) for c in range(NC)]
    # exp(bias)^T per (graph, head): [128, 512, 512]
    biasb = np.exp(np.asarray(attn_bias, f32)).transpose(0, 2, 1).astype(f16)
    mask = np.asarray(node_non_padding_mask).astype(f16)  # [16, 512]

    scale = HD ** -0.5
    in_w = np.asarray(in_w, f32)
    in_b = np.asarray(in_b, f32)
    g1 = np.asarray(ln1_g, f32)
    b1 = np.asarray(ln1_b, f32)
    g2 = np.asarray(ln2_g, f32)
    b2 = np.asarray(ln2_b, f32)
    bq = in_w @ b1 + in_b           # fold LN1 shift into qkv bias
    wq = in_w * g1[None, :]         # fold LN1 gain into qkv weight
    wq = wq.copy()
    bq = bq.copy()
    wq[:D] *= scale
    bq[:D] *= scale
    fc1_w = np.asarray(fc1_w, f32)
    bf1 = fc1_w @ b2 + np.asarray(fc1_b, f32)  # fold LN2 shift
    wf1 = fc1_w * g2[None, :]                  # fold LN2 gain
    shared = {
        "wqkv": np.ascontiguousarray(wq.T.astype(f16)).reshape(KC, 128, 3 * D),
        "bqkv": np.ascontiguousarray(bq.reshape(3 * H, HD).T),
        "wout": np.ascontiguousarray(np.asarray(out_w, f32).T.astype(f16)).reshape(H, HD, D),
        "bout": np.ascontiguousarray(np.asarray(out_b, f32).reshape(KC, 128).T),
        "wfc1": np.ascontiguousarray(wf1.T.astype(f16)).reshape(KC, 128, FFN),
        "bfc1": np.ascontiguousarray(bf1.reshape(FC, 128).T),
        "wfc2": np.ascontiguousarray(np.asarray(fc2_w, f32).T.astype(f16)).reshape(FC, 128, D),
        "bfc2": np.ascontiguousarray(np.asarray(fc2_b, f32).reshape(KC, 128).T),
        "ident": np.eye(128, dtype=f16),
        "ones": np.ones((128, 128), dtype=f16),
    }
    per_core = []
    for c in range(NC):
        m = dict(shared)
        m["xt"] = xt_pc[c]
        m["biasb"] = np.ascontiguousarray(biasb[G * H * c: G * H * (c + 1)])
        m["maskrow"] = np.ascontiguousarray(mask[G * c: G * (c + 1)]).reshape(1, T)
        per_core.append(m)
    return per_core


def postprocess(outs):
    """outs: list of 8 per-core dicts with 'yt' [KC, 128, T] f16 -> [512, 16, 768]"""
    yt = np.stack([o["yt"].astype(np.float32).reshape(D, T) for o in outs])  # [8, 768, 1024]
    y = yt.reshape(NC, D, G, N_NODE).transpose(3, 0, 2, 1).reshape(N_NODE, N_GRAPH, D)
    return np.ascontiguousarray(y)


def run_per_core(per_core):
    r = _get_runner()
    n = NC
    concat_in = [
        np.concatenate([np.asarray(per_core[c][name]) for c in range(n)], axis=0)
        for name in r["in_names"]
    ]
    concat_zeros = [np.zeros((n * z.shape[0], *z.shape[1:]), z.dtype) for z in r["zero_outs"]]
    out_arrs = r["sharded"](*concat_in, *concat_zeros)
    return [
        {name: np.asarray(out_arrs[i]).reshape(n, *r["out_avals"][i].shape)[c]
         for i, name in enumerate(r["out_names"])}
        for c in range(n)
    ]


def kernel(**inputs):
    per_core = prep_inputs(**inputs)
    outs = run_per_core(per_core)
    return postprocess(outs)
